# revision 2
# baseline (speedup 1.0000x reference)
"""CompresSAE topk-masking kernel for 8 Trainium2 NeuronCores.

Pipeline per core (data-parallel over batch, B_core rows):
  A) normalize x rows (scaled by 2048), transpose -> A16 (fp16 x^T) plus
     fp8 DoubleRow residual pairs xdr = [e4m3(ra*32), e4m3(A/8)]
  B) encoder pse = 2048*e via ONE fp16 matmul pass + ONE fp8 DoubleRow
     correction pass per k-tile (12 matmuls/chunk-block instead of 18
     bf16): pse = A16@B16 + (ra*32)@(We/32) + (A16/8)@(rb*8);
     fused per-512-chunk screen: top-8 positive + top-8 negative values
     (+ chunk-local indices) per row -> 1024 candidates/row
  C) top-64-of-candidates per row via 8 rounds of (max8 + match_replace);
     masked candidate values = (cand - zapped) * (+-1/2048), signs+scale
     restored in one multiply
  D) decoder outT = (e_masked @ Wd)^T: rebuild per-chunk dense e_masked
     rows by gpsimd local_scatter, transpose each 128x128 subtile to
     [E,B] layout with XBAR DMA-transpose (no PE/ACT involvement),
     bf16 matmul accumulated in PSUM over E; outT written directly
     (host transposes for free).
"""
import sys

for p in ("/opt/trn_rl_repo", "/root/.axon_site/_ro/trn_rl_repo"):
    if p not in sys.path:
        sys.path.insert(0, p)

import numpy as np

from concourse import bass_utils, tile, bacc
import concourse.mybir as mybir
from concourse.masks import make_identity

dt = mybir.dt
AF = mybir.ActivationFunctionType
P = 128
D = 768
KD = D // P          # 6 contraction tiles
CHUNK = 512          # E-chunk width (= screen subchunk)
NSWEEP = 2           # decoder B-half sweeps (PSUM capacity)
TOPK = 64
ESC = 2048.0         # pse = ESC * e


def build(B_core: int, E: int):
    nblk = B_core // P
    nchunk = E // CHUNK
    bps = nblk // NSWEEP          # blocks per decoder sweep
    ncand = 16 * nchunk           # candidates per row
    EK = CHUNK // P               # 4 E-subtiles per chunk

    nc = bacc.Bacc(trn_type="TRN2", target_bir_lowering=False, debug=False)

    d_x = nc.dram_tensor("x", [B_core, D], dt.float32, kind="ExternalInput").ap()
    d_We = nc.dram_tensor("We", [D, E], dt.float32, kind="ExternalInput").ap()
    d_Wd = nc.dram_tensor("Wd", [E, D], dt.float32, kind="ExternalInput").ap()
    d_outT = nc.dram_tensor("outT", [D, B_core], dt.float32,
                            kind="ExternalOutput").ap()

    with tile.TileContext(nc) as tc:
        with tc.tile_pool(name="consts", bufs=1) as consts, \
             tc.tile_pool(name="live", bufs=1) as live:
            ident_f = consts.tile([P, P], dt.float32)
            make_identity(nc, ident_f)
            # sign/scale pattern over candidate slots: +-1/ESC restores both
            # the sign convention (pos/neg screen halves) and the 1/2048
            # encoder scale in a single multiply
            signpat = consts.tile([P, ncand // 16, 16], dt.float32)
            nc.vector.memset(signpat[:, :, 0:8], 1.0 / ESC)
            nc.vector.memset(signpat[:, :, 8:16], -1.0 / ESC)

            # long-lived per-block arrays
            A16 = [live.tile([P, KD, P], dt.float16, tag=f"A16_{b}", name=f"A16_{b}")
                   for b in range(nblk)]
            xdr = [live.tile([P, KD, 2, P], dt.float8e4, tag=f"xdr{b}", name=f"xdr{b}")
                   for b in range(nblk)]
            cand = [live.tile([P, ncand], dt.float32, tag=f"cand{b}", name=f"cand{b}")
                    for b in range(nblk)]
            lidx = [live.tile([P, ncand], dt.uint16, tag=f"lidx{b}", name=f"lidx{b}")
                    for b in range(nblk)]
            emcand = [live.tile([P, ncand], dt.bfloat16, tag=f"emc{b}", name=f"emc{b}")
                      for b in range(nblk)]

            # ---------------- Phase A: normalize + transpose + split ------
            with tc.tile_pool(name="phA", bufs=2) as phA, \
                 tc.tile_pool(name="psA", bufs=4, space="PSUM") as psA:
                for b in range(nblk):
                    xb = phA.tile([P, D], dt.float32, tag="xb")
                    nc.gpsimd.dma_start(out=xb[:, :], in_=d_x[b * P:(b + 1) * P, :])
                    sq = phA.tile([P, D], dt.float32, tag="sq")
                    ss = phA.tile([P, 1], dt.float32, tag="ss")
                    nc.scalar.activation(sq[:, :], xb[:, :], AF.Square,
                                         accum_out=ss[:, :])
                    nrm = phA.tile([P, 1], dt.float32, tag="nrm")
                    nc.scalar.activation(nrm[:, :], ss[:, :], AF.Sqrt)
                    rn = phA.tile([P, 1], dt.float32, tag="rn")
                    nc.vector.reciprocal(rn[:, :], nrm[:, :])
                    rs = phA.tile([P, 1], dt.float32, tag="rs")
                    nc.vector.tensor_scalar_mul(rs[:, :], rn[:, :], ESC)
                    xnb = phA.tile([P, D], dt.float32, tag="xnb")
                    nc.scalar.activation(xnb[:, :], xb[:, :], AF.Copy,
                                         scale=rs[:, :])
                    # transpose 6 [128,128] tiles; emit fp16 + fp8 operands
                    for g in range(2):      # two psum packs of 3 tiles
                        pk = psA.tile([P, 3 * P], dt.float32, tag="psA")
                        for j in range(3):
                            k = g * 3 + j
                            nc.tensor.transpose(pk[:, j * P:(j + 1) * P],
                                                xnb[:, k * P:(k + 1) * P],
                                                ident_f[:, :])
                        a_sl = A16[b][:, g * 3:(g + 1) * 3, :]
                        nc.scalar.copy(out=a_sl, in_=pk[:, :])
                        ra32 = phA.tile([P, 3 * P], dt.float32, tag="ra32")
                        nc.vector.tensor_sub(out=ra32[:, :], in0=pk[:, :],
                                             in1=a_sl)
                        nc.scalar.activation(
                            xdr[b][:, g * 3:(g + 1) * 3, 0, :], ra32[:, :],
                            AF.Copy, scale=32.0)
                        nc.scalar.activation(
                            xdr[b][:, g * 3:(g + 1) * 3, 1, :], a_sl,
                            AF.Copy, scale=0.125)

            # ---------------- Phase B: encoder + fused screen -------------
            with tc.tile_pool(name="wstage", bufs=2) as wstage, \
                 tc.tile_pool(name="whl", bufs=2) as whl, \
                 tc.tile_pool(name="scr", bufs=4) as scr, \
                 tc.tile_pool(name="psB", bufs=1, space="PSUM") as psB:
                pse = [psB.tile([P, CHUNK], dt.float32, tag=f"pse{b}", name=f"pse{b}")
                       for b in range(nblk)]
                for c in range(nchunk):
                    wf = wstage.tile([P, KD, CHUNK], dt.float32, tag="wf")
                    nc.gpsimd.dma_start(
                        out=wf[:, :, :],
                        in_=d_We[:, c * CHUNK:(c + 1) * CHUNK].rearrange(
                            "(k p) n -> p k n", p=P))
                    B16 = whl.tile([P, KD, CHUNK], dt.float16, tag="B16")
                    nc.scalar.copy(out=B16[:, :, :], in_=wf[:, :, :])
                    rb32 = whl.tile([P, KD, CHUNK], dt.float32, tag="rb32")
                    nc.vector.tensor_sub(out=rb32[:, :, :], in0=wf[:, :, :],
                                         in1=B16[:, :, :])
                    wdr = whl.tile([P, KD, 2, CHUNK], dt.float8e5, tag="wdr")
                    nc.scalar.activation(wdr[:, :, 0, :], wf[:, :, :],
                                         AF.Copy, scale=1.0 / 32.0)
                    nc.scalar.activation(wdr[:, :, 1, :], rb32[:, :, :],
                                         AF.Copy, scale=8.0)
                    for b in range(nblk):
                        for k in range(KD):
                            nc.tensor.matmul(
                                pse[b][:, :], A16[b][:, k, :], B16[:, k, :],
                                start=(k == 0), stop=False)
                        for k in range(KD):
                            nc.tensor.matmul(
                                pse[b][:, :], xdr[b][:, k, :, :],
                                wdr[:, k, :, :],
                                start=False, stop=(k == KD - 1),
                                perf_mode=mybir.MatmulPerfMode.DoubleRow)
                        # negated copy for the negative-side screen
                        en = scr.tile([P, CHUNK], dt.float32, tag="en")
                        nc.scalar.activation(en[:, :], pse[b][:, :],
                                             AF.Copy, scale=-1.0)
                        # screens: top-8 of e (pos) and of -e (neg)
                        nc.vector.max(out=cand[b][:, 16 * c:16 * c + 8],
                                      in_=pse[b][:, :])
                        nc.vector.max_index(out=lidx[b][:, 16 * c:16 * c + 8],
                                            in_max=cand[b][:, 16 * c:16 * c + 8],
                                            in_values=pse[b][:, :])
                        nc.vector.max(out=cand[b][:, 16 * c + 8:16 * c + 16],
                                      in_=en[:, :])
                        nc.vector.max_index(
                            out=lidx[b][:, 16 * c + 8:16 * c + 16],
                            in_max=cand[b][:, 16 * c + 8:16 * c + 16],
                            in_values=en[:, :])

            # ---------------- Phase C helper: top-64 of candidates --------
            def emit_phaseC(phC, b):
                s1 = phC.tile([P, ncand], dt.float32, tag="s1", name=f"s1_{b}")
                s2 = phC.tile([P, ncand], dt.float32, tag="s2", name=f"s2_{b}")
                cur = cand[b]
                dst = s1
                for r in range(TOPK // 8):
                    v8 = phC.tile([P, 8], dt.float32, tag="v8", name=f"v8_{b}_{r}")
                    nc.vector.max(out=v8[:, :], in_=cur[:, :])
                    nc.vector.match_replace(out=dst[:, :],
                                            in_to_replace=v8[:, :],
                                            in_values=cur[:, :],
                                            imm_value=0.0)
                    cur, dst = dst, (s2 if dst is s1 else s1)
                dd = phC.tile([P, ncand], dt.float32, tag="dd", name=f"dd_{b}")
                nc.vector.tensor_sub(out=dd[:, :], in0=cand[b][:, :],
                                     in1=cur[:, :])
                nc.vector.tensor_mul(
                    out=emcand[b][:, :], in0=dd[:, :],
                    in1=signpat[:, :, :].rearrange("p a b -> p (a b)"))

            # ---------------- Phase D: decoder (with interleaved C) -------
            with tc.tile_pool(name="phC", bufs=2) as phC, \
                 tc.tile_pool(name="wdstage", bufs=2) as wdstage, \
                 tc.tile_pool(name="wdh", bufs=2) as wdhp, \
                 tc.tile_pool(name="emc", bufs=6) as emcp, \
                 tc.tile_pool(name="rhs", bufs=3) as rhsp, \
                 tc.tile_pool(name="tail", bufs=2) as tailp, \
                 tc.tile_pool(name="psD", bufs=1, space="PSUM") as psD:
                for sw in range(NSWEEP):
                    for bi in range(bps):
                        emit_phaseC(phC, sw * bps + bi)
                    pso = [psD.tile([P, bps * P], dt.float32, tag=f"pso{m}", name=f"pso{m}_{sw}")
                           for m in range(KD)]
                    for c in range(nchunk):
                        wdf = wdstage.tile([P, EK, D], dt.float32, tag="wdf")
                        nc.gpsimd.dma_start(
                            out=wdf[:, :, :],
                            in_=d_Wd[c * CHUNK:(c + 1) * CHUNK, :].rearrange(
                                "(k p) n -> p k n", p=P))
                        wdh = wdhp.tile([P, EK, D], dt.bfloat16, tag="wdh")
                        nc.vector.tensor_copy(out=wdh[:, :, :], in_=wdf[:, :, :])
                        # rebuild dense masked-e rows for this chunk, then
                        # XBAR-DMA-transpose each 128x128 subtile to [E,B]
                        rtall = rhsp.tile([P, EK, bps, P], dt.bfloat16,
                                          tag="rtall", name=f"rt_{sw}_{c}")
                        for bi in range(bps):
                            b = sw * bps + bi
                            em = emcp.tile([P, CHUNK], dt.bfloat16, tag="em")
                            nc.gpsimd.local_scatter(
                                em[:, :],
                                emcand[b][:, 16 * c:16 * c + 16],
                                lidx[b][:, 16 * c:16 * c + 16].bitcast(dt.int16),
                                channels=P, num_elems=CHUNK, num_idxs=16)
                            for es in range(EK):
                                nc.sync.dma_start_transpose(
                                    out=rtall[:, es, bi, :],
                                    in_=em[:, es * P:(es + 1) * P])
                        for m in range(KD):
                            for es in range(EK):
                                nc.tensor.matmul(
                                    pso[m][:, :],
                                    wdh[:, es, m * P:(m + 1) * P],
                                    rtall[:, es, :, :],
                                    start=(c == 0 and es == 0),
                                    stop=(c == nchunk - 1 and es == EK - 1))
                    # tail: outT slices straight from PSUM via one copy
                    for m in range(KD):
                        ot = tailp.tile([P, bps * P], dt.float32, tag="ot",
                                        name=f"ot{m}_{sw}")
                        nc.scalar.copy(out=ot[:, :], in_=pso[m][:, :])
                        nc.gpsimd.dma_start(
                            out=d_outT[m * P:(m + 1) * P,
                                       sw * bps * P:(sw + 1) * bps * P],
                            in_=ot[:, :])

    nc.compile()
    return nc


_CACHE = {}


def _get(B_core, E):
    key = (B_core, E)
    if key not in _CACHE:
        _CACHE[key] = build(B_core, E)
    return _CACHE[key]


def kernel(x, encoder_w, encoder_b, decoder_w, k, n_cores=8):
    x = np.ascontiguousarray(np.asarray(x, dtype=np.float32))
    We = np.ascontiguousarray(np.asarray(encoder_w, dtype=np.float32))
    Wd = np.ascontiguousarray(np.asarray(decoder_w, dtype=np.float32))
    b = np.asarray(encoder_b)
    assert int(np.asarray(k)) == TOPK, f"kernel compiled for k={TOPK}"
    assert not np.any(b), "nonzero encoder_b not supported"
    B, Dd = x.shape
    E = We.shape[1]
    assert Dd == D and B % n_cores == 0
    B_core = B // n_cores

    nc = _get(B_core, E)
    in_maps = [{"x": x[i * B_core:(i + 1) * B_core], "We": We, "Wd": Wd}
               for i in range(n_cores)]
    res = bass_utils.run_bass_kernel_spmd(nc, in_maps,
                                          core_ids=list(range(n_cores)))
    return np.concatenate(
        [np.ascontiguousarray(res.results[i]["outT"].T)
         for i in range(n_cores)], axis=0)


# revision 4
# speedup vs baseline: 1.9758x; 1.9758x over previous
"""CompresSAE topk-masking kernel for 8 Trainium2 NeuronCores.

Pipeline per core (data-parallel over batch, B_core rows):
  A) normalize x rows (scaled by 2048), transpose -> A16 (fp16 x^T) plus
     fp8 DoubleRow residual pairs xdr = [e4m3(ra*32), e4m3(A/8)]
  B) encoder pse = 2048*e via ONE fp16 matmul pass + ONE fp8 DoubleRow
     correction pass per k-tile (12 matmuls/chunk-block instead of 18
     bf16): pse = A16@B16 + (ra*32)@(We/32) + (A16/8)@(rb*8);
     fused per-512-chunk screen: top-8 positive + top-8 negative values
     (+ chunk-local indices) per row -> 1024 candidates/row
  C) top-64-of-candidates per row via 8 rounds of (max8 + match_replace);
     masked candidate values = (cand - zapped) * (+-1/2048), signs+scale
     restored in one multiply
  D) decoder outT = (e_masked @ Wd)^T: rebuild per-chunk dense e_masked
     rows by gpsimd local_scatter, transpose each 128x128 subtile to
     [E,B] layout with XBAR DMA-transpose (no PE/ACT involvement),
     bf16 matmul accumulated in PSUM over E; outT written directly
     (host transposes for free).
"""
import sys

for p in ("/opt/trn_rl_repo", "/root/.axon_site/_ro/trn_rl_repo"):
    if p not in sys.path:
        sys.path.insert(0, p)

import numpy as np

from concourse import bass_utils, tile, bacc
import concourse.mybir as mybir
from concourse.masks import make_identity

dt = mybir.dt
AF = mybir.ActivationFunctionType
P = 128
D = 768
KD = D // P          # 6 contraction tiles
CHUNK = 512          # E-chunk width (= screen subchunk)
NSWEEP = 2           # decoder B-half sweeps (PSUM capacity)
TOPK = 64
ESC = 2048.0         # pse = ESC * e


def build(B_core: int, E: int):
    nblk = B_core // P
    nchunk = E // CHUNK
    bps = nblk // NSWEEP          # blocks per decoder sweep
    ncand = 16 * nchunk           # candidates per row
    EK = CHUNK // P               # 4 E-subtiles per chunk

    nc = bacc.Bacc(trn_type="TRN2", target_bir_lowering=False, debug=False)

    d_x = nc.dram_tensor("x", [B_core, D], dt.float32, kind="ExternalInput").ap()
    d_We = nc.dram_tensor("We", [D, E], dt.float32, kind="ExternalInput").ap()
    d_Wd = nc.dram_tensor("Wd", [E, D], dt.float32, kind="ExternalInput").ap()
    d_outT = nc.dram_tensor("outT", [D, B_core], dt.float32,
                            kind="ExternalOutput").ap()

    with tile.TileContext(nc) as tc:
        with tc.tile_pool(name="consts", bufs=1) as consts, \
             tc.tile_pool(name="live", bufs=1) as live:
            ident_f = consts.tile([P, P], dt.float32)
            make_identity(nc, ident_f)
            ident_b = consts.tile([P, P], dt.bfloat16)
            make_identity(nc, ident_b)
            # sign/scale pattern over candidate slots: +-1/ESC restores both
            # the sign convention (pos/neg screen halves) and the 1/2048
            # encoder scale in a single multiply
            signpat = consts.tile([P, ncand // 16, 16], dt.float32)
            nc.vector.memset(signpat[:, :, 0:8], 1.0 / ESC)
            nc.vector.memset(signpat[:, :, 8:16], -1.0 / ESC)

            # long-lived per-block arrays
            A16 = [live.tile([P, KD, P], dt.float16, tag=f"A16_{b}", name=f"A16_{b}")
                   for b in range(nblk)]
            xdr = [live.tile([P, KD, 2, P], dt.float8e4, tag=f"xdr{b}", name=f"xdr{b}")
                   for b in range(nblk)]
            cand = [live.tile([P, ncand], dt.float32, tag=f"cand{b}", name=f"cand{b}")
                    for b in range(nblk)]
            lidx = [live.tile([P, ncand], dt.uint16, tag=f"lidx{b}", name=f"lidx{b}")
                    for b in range(nblk)]
            emcand = [live.tile([P, ncand], dt.bfloat16, tag=f"emc{b}", name=f"emc{b}")
                      for b in range(nblk)]

            # ---------------- Phase A: normalize + transpose + split ------
            with tc.tile_pool(name="phA", bufs=2) as phA, \
                 tc.tile_pool(name="psA", bufs=4, space="PSUM") as psA:
                for b in range(nblk):
                    xb = phA.tile([P, D], dt.float32, tag="xb")
                    nc.gpsimd.dma_start(out=xb[:, :], in_=d_x[b * P:(b + 1) * P, :])
                    sq = phA.tile([P, D], dt.float32, tag="sq")
                    ss = phA.tile([P, 1], dt.float32, tag="ss")
                    nc.scalar.activation(sq[:, :], xb[:, :], AF.Square,
                                         accum_out=ss[:, :])
                    nrm = phA.tile([P, 1], dt.float32, tag="nrm")
                    nc.scalar.activation(nrm[:, :], ss[:, :], AF.Sqrt)
                    rn = phA.tile([P, 1], dt.float32, tag="rn")
                    nc.vector.reciprocal(rn[:, :], nrm[:, :])
                    rs = phA.tile([P, 1], dt.float32, tag="rs")
                    nc.vector.tensor_scalar_mul(rs[:, :], rn[:, :], ESC)
                    xnb = phA.tile([P, D], dt.float32, tag="xnb")
                    nc.scalar.activation(xnb[:, :], xb[:, :], AF.Copy,
                                         scale=rs[:, :])
                    # transpose 6 [128,128] tiles; emit fp16 + fp8 operands
                    for g in range(2):      # two psum packs of 3 tiles
                        pk = psA.tile([P, 3 * P], dt.float32, tag="psA")
                        for j in range(3):
                            k = g * 3 + j
                            nc.tensor.transpose(pk[:, j * P:(j + 1) * P],
                                                xnb[:, k * P:(k + 1) * P],
                                                ident_f[:, :])
                        a_sl = A16[b][:, g * 3:(g + 1) * 3, :]
                        nc.scalar.copy(out=a_sl, in_=pk[:, :])
                        ra32 = phA.tile([P, 3 * P], dt.float32, tag="ra32")
                        nc.vector.tensor_sub(out=ra32[:, :], in0=pk[:, :],
                                             in1=a_sl)
                        nc.scalar.activation(
                            xdr[b][:, g * 3:(g + 1) * 3, 0, :], ra32[:, :],
                            AF.Copy, scale=32.0)
                        nc.scalar.activation(
                            xdr[b][:, g * 3:(g + 1) * 3, 1, :], a_sl,
                            AF.Copy, scale=0.125)

            # ---------------- Phase B: encoder + fused screen -------------
            with tc.tile_pool(name="wstage", bufs=2) as wstage, \
                 tc.tile_pool(name="whl", bufs=2) as whl, \
                 tc.tile_pool(name="scr", bufs=4) as scr, \
                 tc.tile_pool(name="psB", bufs=1, space="PSUM") as psB:
                pse = [psB.tile([P, CHUNK], dt.float32, tag=f"pse{b}", name=f"pse{b}")
                       for b in range(nblk)]
                for c in range(nchunk):
                    wf = wstage.tile([P, KD, CHUNK], dt.float32, tag="wf")
                    nc.gpsimd.dma_start(
                        out=wf[:, :, :],
                        in_=d_We[:, c * CHUNK:(c + 1) * CHUNK].rearrange(
                            "(k p) n -> p k n", p=P))
                    B16 = whl.tile([P, KD, CHUNK], dt.float16, tag="B16")
                    nc.scalar.copy(out=B16[:, :, :], in_=wf[:, :, :])
                    rb32 = whl.tile([P, KD, CHUNK], dt.float32, tag="rb32")
                    nc.vector.tensor_sub(out=rb32[:, :, :], in0=wf[:, :, :],
                                         in1=B16[:, :, :])
                    wdr = whl.tile([P, KD, 2, CHUNK], dt.float8e5, tag="wdr")
                    nc.scalar.activation(wdr[:, :, 0, :], wf[:, :, :],
                                         AF.Copy, scale=1.0 / 32.0)
                    nc.scalar.activation(wdr[:, :, 1, :], rb32[:, :, :],
                                         AF.Copy, scale=8.0)
                    for b in range(nblk):
                        for k in range(KD):
                            nc.tensor.matmul(
                                pse[b][:, :], A16[b][:, k, :], B16[:, k, :],
                                start=(k == 0), stop=False)
                        for k in range(KD):
                            nc.tensor.matmul(
                                pse[b][:, :], xdr[b][:, k, :, :],
                                wdr[:, k, :, :],
                                start=False, stop=(k == KD - 1),
                                perf_mode=mybir.MatmulPerfMode.DoubleRow)
                        # negated copy for the negative-side screen
                        en = scr.tile([P, CHUNK], dt.float32, tag="en")
                        nc.scalar.activation(en[:, :], pse[b][:, :],
                                             AF.Copy, scale=-1.0)
                        # screens: top-8 of e (pos) and of -e (neg)
                        nc.vector.max(out=cand[b][:, 16 * c:16 * c + 8],
                                      in_=pse[b][:, :])
                        nc.vector.max_index(out=lidx[b][:, 16 * c:16 * c + 8],
                                            in_max=cand[b][:, 16 * c:16 * c + 8],
                                            in_values=pse[b][:, :])
                        nc.vector.max(out=cand[b][:, 16 * c + 8:16 * c + 16],
                                      in_=en[:, :])
                        nc.vector.max_index(
                            out=lidx[b][:, 16 * c + 8:16 * c + 16],
                            in_max=cand[b][:, 16 * c + 8:16 * c + 16],
                            in_values=en[:, :])

            # ---------------- Phase C helper: top-64 of candidates --------
            def emit_phaseC(phC, b):
                s1 = phC.tile([P, ncand], dt.float32, tag="s1", name=f"s1_{b}")
                s2 = phC.tile([P, ncand], dt.float32, tag="s2", name=f"s2_{b}")
                cur = cand[b]
                dst = s1
                for r in range(TOPK // 8):
                    v8 = phC.tile([P, 8], dt.float32, tag="v8", name=f"v8_{b}_{r}")
                    nc.vector.max(out=v8[:, :], in_=cur[:, :])
                    nc.vector.match_replace(out=dst[:, :],
                                            in_to_replace=v8[:, :],
                                            in_values=cur[:, :],
                                            imm_value=0.0)
                    cur, dst = dst, (s2 if dst is s1 else s1)
                dd = phC.tile([P, ncand], dt.float32, tag="dd", name=f"dd_{b}")
                nc.vector.tensor_sub(out=dd[:, :], in0=cand[b][:, :],
                                     in1=cur[:, :])
                nc.vector.tensor_mul(
                    out=emcand[b][:, :], in0=dd[:, :],
                    in1=signpat[:, :, :].rearrange("p a b -> p (a b)"))

            # ---------------- Phase D: decoder (with interleaved C) -------
            with tc.tile_pool(name="phC", bufs=2) as phC, \
                 tc.tile_pool(name="wdstage", bufs=2) as wdstage, \
                 tc.tile_pool(name="wdh", bufs=2) as wdhp, \
                 tc.tile_pool(name="emc", bufs=6) as emcp, \
                 tc.tile_pool(name="rhs", bufs=3) as rhsp, \
                 tc.tile_pool(name="tail", bufs=2) as tailp, \
                 tc.tile_pool(name="psD", bufs=1, space="PSUM") as psD, \
                 tc.tile_pool(name="psT", bufs=2, space="PSUM") as psT:
                for sw in range(NSWEEP):
                    for bi in range(bps):
                        emit_phaseC(phC, sw * bps + bi)
                    pso = [psD.tile([P, bps * P], dt.float32, tag=f"pso{m}", name=f"pso{m}_{sw}")
                           for m in range(KD)]
                    for c in range(nchunk):
                        wdf = wdstage.tile([P, EK, D], dt.float32, tag="wdf")
                        nc.gpsimd.dma_start(
                            out=wdf[:, :, :],
                            in_=d_Wd[c * CHUNK:(c + 1) * CHUNK, :].rearrange(
                                "(k p) n -> p k n", p=P))
                        wdh = wdhp.tile([P, EK, D], dt.bfloat16, tag="wdh")
                        nc.vector.tensor_copy(out=wdh[:, :, :], in_=wdf[:, :, :])
                        # rebuild dense masked-e rows for this chunk + transpose
                        rhs = []
                        for es in range(EK):
                            pk = psT.tile([P, bps * P], dt.bfloat16, tag="psT",
                                          name=f"psT{sw}_{c}_{es}")
                            rhs.append((es, pk))
                        for bi in range(bps):
                            b = sw * bps + bi
                            em = emcp.tile([P, CHUNK], dt.bfloat16, tag="em")
                            nc.gpsimd.local_scatter(
                                em[:, :],
                                emcand[b][:, 16 * c:16 * c + 16],
                                lidx[b][:, 16 * c:16 * c + 16].bitcast(dt.int16),
                                channels=P, num_elems=CHUNK, num_idxs=16)
                            for (es, pk) in rhs:
                                nc.tensor.transpose(
                                    pk[:, bi * P:(bi + 1) * P],
                                    em[:, es * P:(es + 1) * P],
                                    ident_b[:, :])
                        rr = []
                        for (es, pk) in rhs:
                            rt = rhsp.tile([P, bps * P], dt.bfloat16,
                                           tag=f"rt{es}", name=f"rt{es}_{sw}_{c}")
                            nc.scalar.copy(out=rt[:, :], in_=pk[:, :])
                            rr.append(rt)
                        for m in range(KD):
                            for es in range(EK):
                                nc.tensor.matmul(
                                    pso[m][:, :],
                                    wdh[:, es, m * P:(m + 1) * P],
                                    rr[es][:, :],
                                    start=(c == 0 and es == 0),
                                    stop=(c == nchunk - 1 and es == EK - 1))
                    # tail: outT slices straight from PSUM via one copy
                    for m in range(KD):
                        ot = tailp.tile([P, bps * P], dt.float32, tag="ot",
                                        name=f"ot{m}_{sw}")
                        nc.scalar.copy(out=ot[:, :], in_=pso[m][:, :])
                        nc.gpsimd.dma_start(
                            out=d_outT[m * P:(m + 1) * P,
                                       sw * bps * P:(sw + 1) * bps * P],
                            in_=ot[:, :])

    nc.compile()
    return nc


_CACHE = {}


def _get(B_core, E):
    key = (B_core, E)
    if key not in _CACHE:
        _CACHE[key] = build(B_core, E)
    return _CACHE[key]


def kernel(x, encoder_w, encoder_b, decoder_w, k, n_cores=8):
    x = np.ascontiguousarray(np.asarray(x, dtype=np.float32))
    We = np.ascontiguousarray(np.asarray(encoder_w, dtype=np.float32))
    Wd = np.ascontiguousarray(np.asarray(decoder_w, dtype=np.float32))
    b = np.asarray(encoder_b)
    assert int(np.asarray(k)) == TOPK, f"kernel compiled for k={TOPK}"
    assert not np.any(b), "nonzero encoder_b not supported"
    B, Dd = x.shape
    E = We.shape[1]
    assert Dd == D and B % n_cores == 0
    B_core = B // n_cores

    nc = _get(B_core, E)
    in_maps = [{"x": x[i * B_core:(i + 1) * B_core], "We": We, "Wd": Wd}
               for i in range(n_cores)]
    res = bass_utils.run_bass_kernel_spmd(nc, in_maps,
                                          core_ids=list(range(n_cores)))
    return np.concatenate(
        [np.ascontiguousarray(res.results[i]["outT"].T)
         for i in range(n_cores)], axis=0)


# revision 10
# speedup vs baseline: 1.9977x; 1.0111x over previous
"""CompresSAE topk-masking kernel for 8 Trainium2 NeuronCores.

Pipeline per core (data-parallel over batch, B_core rows):
  A) normalize x rows (scaled by 2048), transpose -> A16 (fp16 x^T) plus
     fp8 DoubleRow residual pairs xdr = [e4m3(ra*32), e4m3(A16/8)]
  B) encoder pse = 2048*e via ONE fp16 matmul pass + ONE fp8 DoubleRow
     correction pass per k-tile (12 matmuls/chunk-block instead of 18
     bf16): pse = A16@B16 + (ra*32)@(We/32) + (A16/8)@(rb*8);
     fused per-512-chunk screen: top-8 positive + top-8 negative values
     (+ chunk-local indices) per row -> 1024 candidates/row
  C) top-64-of-candidates per row via 8 rounds of (max8 + match_replace);
     emitted inline right after each block's last screen so it overlaps
     the encoder tail; masked values = (cand - zapped) * (+-1/2048)
  D) decoder outT = (e_masked @ Wd)^T: rebuild per-chunk dense e_masked
     rows by gpsimd local_scatter, PE-transpose to [E,B] tiles, bf16
     matmul accumulated in PSUM over E; outT written directly (host
     transposes for free).

All pools live in one scope (no inter-phase barriers); phase A uses the
encoder PSUM banks as transpose scratch; Wd chunk 0/1 are prefetched
during the encoder.
"""
import sys

for p in ("/opt/trn_rl_repo", "/root/.axon_site/_ro/trn_rl_repo"):
    if p not in sys.path:
        sys.path.insert(0, p)

import numpy as np

from concourse import bass_utils, tile, bacc
import concourse.mybir as mybir
from concourse.masks import make_identity

dt = mybir.dt
AF = mybir.ActivationFunctionType
P = 128
D = 768
KD = D // P          # 6 contraction tiles
CHUNK = 512          # E-chunk width (= screen subchunk)
NSWEEP = 2           # decoder B-half sweeps (PSUM capacity)
TOPK = 64
ESC = 2048.0         # pse = ESC * e


def build(B_core: int, E: int):
    nblk = B_core // P
    nchunk = E // CHUNK
    bps = nblk // NSWEEP          # blocks per decoder sweep
    ncand = 16 * nchunk           # candidates per row
    EK = CHUNK // P               # 4 E-subtiles per chunk

    nc = bacc.Bacc(trn_type="TRN2", target_bir_lowering=False, debug=False)

    d_x = nc.dram_tensor("x", [B_core, D], dt.float32, kind="ExternalInput").ap()
    d_We = nc.dram_tensor("We", [D, E], dt.float32, kind="ExternalInput").ap()
    d_Wd = nc.dram_tensor("Wd", [E, D], dt.float32, kind="ExternalInput").ap()
    d_outT = nc.dram_tensor("outT", [D, B_core], dt.float32,
                            kind="ExternalOutput").ap()

    with tile.TileContext(nc) as tc:
        with tc.tile_pool(name="consts", bufs=1) as consts, \
             tc.tile_pool(name="live", bufs=1) as live, \
             tc.tile_pool(name="phA", bufs=2) as phA, \
             tc.tile_pool(name="wstage", bufs=2) as wstage, \
             tc.tile_pool(name="whl", bufs=2) as whl, \
             tc.tile_pool(name="rbp", bufs=1) as rbp, \
             tc.tile_pool(name="scr", bufs=4) as scr, \
             tc.tile_pool(name="phC", bufs=1) as phC, \
             tc.tile_pool(name="wdstage", bufs=2) as wdstage, \
             tc.tile_pool(name="wdh", bufs=2) as wdhp, \
             tc.tile_pool(name="emc", bufs=4) as emcp, \
             tc.tile_pool(name="rhs", bufs=2) as rhsp, \
             tc.tile_pool(name="tail", bufs=2) as tailp:
            ident_f = consts.tile([P, P], dt.float32)
            make_identity(nc, ident_f)
            ident_b = consts.tile([P, P], dt.bfloat16)
            make_identity(nc, ident_b)
            # sign/scale pattern over candidate slots: +-1/ESC restores both
            # the pos/neg sign convention and the 1/2048 encoder scale
            signpat = consts.tile([P, ncand // 16, 16], dt.float32)
            nc.vector.memset(signpat[:, :, 0:8], 1.0 / ESC)
            nc.vector.memset(signpat[:, :, 8:16], -1.0 / ESC)

            # long-lived per-block arrays
            A16 = [live.tile([P, KD, P], dt.float16, tag=f"A16_{b}", name=f"A16_{b}")
                   for b in range(nblk)]
            xdr = [live.tile([P, KD, 2, P], dt.float8e4, tag=f"xdr{b}", name=f"xdr{b}")
                   for b in range(nblk)]
            cand = [live.tile([P, ncand], dt.float32, tag=f"cand{b}", name=f"cand{b}")
                    for b in range(nblk)]
            lidx = [live.tile([P, ncand], dt.uint16, tag=f"lidx{b}", name=f"lidx{b}")
                    for b in range(nblk)]
            emcand = [live.tile([P, ncand], dt.bfloat16, tag=f"emc{b}", name=f"emc{b}")
                      for b in range(nblk)]

            # encoder PSUM banks (also phase-A transpose scratch)
            psB_cm = tc.tile_pool(name="psB", bufs=1, space="PSUM")
            psB = psB_cm.__enter__()
            pse = [psB.tile([P, CHUNK], dt.float32, tag=f"pse{b}", name=f"pse{b}")
                   for b in range(nblk)]

            # ---------------- Phase A: normalize + transpose + split ------
            for b in range(nblk):
                xb = phA.tile([P, D], dt.float32, tag="xb")
                nc.gpsimd.dma_start(out=xb[:, :], in_=d_x[b * P:(b + 1) * P, :])
                xnb = phA.tile([P, D], dt.float32, tag="xnb")
                ss = phA.tile([P, 1], dt.float32, tag="ss")
                # square into xnb as scratch; only the row-sum accum is used
                nc.scalar.activation(xnb[:, :], xb[:, :], AF.Square,
                                     accum_out=ss[:, :])
                nrm = phA.tile([P, 1], dt.float32, tag="nrm")
                nc.scalar.activation(nrm[:, :], ss[:, :], AF.Sqrt)
                rn = phA.tile([P, 1], dt.float32, tag="rn")
                nc.vector.reciprocal(rn[:, :], nrm[:, :])
                rs = phA.tile([P, 1], dt.float32, tag="rs")
                nc.vector.tensor_scalar_mul(rs[:, :], rn[:, :], ESC)
                nc.scalar.activation(xnb[:, :], xb[:, :], AF.Copy,
                                     scale=rs[:, :])
                # transpose 6 [128,128] tiles via the pse[b] bank as scratch
                for g in range(2):
                    pk = pse[b][:, 0:3 * P]
                    for j in range(3):
                        k = g * 3 + j
                        nc.tensor.transpose(pk[:, j * P:(j + 1) * P],
                                            xnb[:, k * P:(k + 1) * P],
                                            ident_f[:, :])
                    a_sl = A16[b][:, g * 3:(g + 1) * 3, :]
                    nc.vector.tensor_copy(out=a_sl, in_=pk)
                    ra32 = phA.tile([P, 3 * P], dt.float32, tag="ra32")
                    nc.vector.tensor_sub(out=ra32[:, :], in0=pk, in1=a_sl)
                    nc.scalar.activation(
                        xdr[b][:, g * 3:(g + 1) * 3, 0, :], ra32[:, :],
                        AF.Copy, scale=32.0)
                    nc.scalar.activation(
                        xdr[b][:, g * 3:(g + 1) * 3, 1, :], a_sl,
                        AF.Copy, scale=0.125)

            # -------- Wd prefetch for decoder chunks 0/1 (no deps) --------
            wdf_pre = []
            for c in range(2):
                wdf = wdstage.tile([P, EK, D], dt.float32, tag="wdf",
                                   name=f"wdfpre{c}")
                nc.gpsimd.dma_start(
                    out=wdf[:, :, :],
                    in_=d_Wd[c * CHUNK:(c + 1) * CHUNK, :].rearrange(
                        "(k p) n -> p k n", p=P))
                wdh = wdhp.tile([P, EK, D], dt.bfloat16, tag="wdh",
                                name=f"wdhpre{c}")
                nc.vector.tensor_copy(out=wdh[:, :, :], in_=wdf[:, :, :])
                wdf_pre.append(wdh)

            # ---------------- Phase C helper: top-64 of candidates --------
            def emit_phaseC(b):
                s1 = phC.tile([P, ncand], dt.float32, tag="s1", name=f"s1_{b}")
                s2 = phC.tile([P, ncand], dt.float32, tag="s2", name=f"s2_{b}")
                cur = cand[b]
                dst = s1
                for r in range(TOPK // 8):
                    v8 = phC.tile([P, 8], dt.float32, tag="v8", name=f"v8_{b}_{r}")
                    nc.vector.max(out=v8[:, :], in_=cur[:, :])
                    nc.vector.match_replace(out=dst[:, :],
                                            in_to_replace=v8[:, :],
                                            in_values=cur[:, :],
                                            imm_value=0.0)
                    cur, dst = dst, (s2 if dst is s1 else s1)
                dd = phC.tile([P, ncand], dt.float32, tag="dd", name=f"dd_{b}")
                nc.vector.tensor_sub(out=dd[:, :], in0=cand[b][:, :],
                                     in1=cur[:, :])
                nc.vector.tensor_mul(
                    out=emcand[b][:, :], in0=dd[:, :],
                    in1=signpat[:, :, :].rearrange("p a b -> p (a b)"))

            # ---------------- Phase B: encoder + fused screen -------------
            for c in range(nchunk):
                wf = wstage.tile([P, KD, CHUNK], dt.float32, tag="wf")
                nc.gpsimd.dma_start(
                    out=wf[:, :, :],
                    in_=d_We[:, c * CHUNK:(c + 1) * CHUNK].rearrange(
                        "(k p) n -> p k n", p=P))
                B16 = whl.tile([P, KD, CHUNK], dt.float16, tag="B16")
                nc.scalar.copy(out=B16[:, :, :], in_=wf[:, :, :])
                rb32 = rbp.tile([P, KD, CHUNK], dt.float32, tag="rb32")
                nc.vector.tensor_sub(out=rb32[:, :, :], in0=wf[:, :, :],
                                     in1=B16[:, :, :])
                wdr = whl.tile([P, KD, 2, CHUNK], dt.float8e5, tag="wdr")
                nc.scalar.activation(wdr[:, :, 0, :], wf[:, :, :],
                                     AF.Copy, scale=1.0 / 32.0)
                nc.scalar.activation(wdr[:, :, 1, :], rb32[:, :, :],
                                     AF.Copy, scale=8.0)
                for b in range(nblk):
                    for k in range(KD):
                        nc.tensor.matmul(
                            pse[b][:, :], A16[b][:, k, :], B16[:, k, :],
                            start=(k == 0), stop=False)
                    for k in range(KD):
                        nc.tensor.matmul(
                            pse[b][:, :], xdr[b][:, k, :, :],
                            wdr[:, k, :, :],
                            start=False, stop=(k == KD - 1),
                            perf_mode=mybir.MatmulPerfMode.DoubleRow)
                    # negated copy for the negative-side screen
                    en = scr.tile([P, CHUNK], dt.float32, tag="en")
                    nc.scalar.activation(en[:, :], pse[b][:, :],
                                         AF.Copy, scale=-1.0)
                    # screens: top-8 of e (pos) and of -e (neg)
                    nc.vector.max(out=cand[b][:, 16 * c:16 * c + 8],
                                  in_=pse[b][:, :])
                    nc.vector.max_index(out=lidx[b][:, 16 * c:16 * c + 8],
                                        in_max=cand[b][:, 16 * c:16 * c + 8],
                                        in_values=pse[b][:, :])
                    nc.vector.max(out=cand[b][:, 16 * c + 8:16 * c + 16],
                                  in_=en[:, :])
                    nc.vector.max_index(
                        out=lidx[b][:, 16 * c + 8:16 * c + 16],
                        in_max=cand[b][:, 16 * c + 8:16 * c + 16],
                        in_values=en[:, :])
                    if c == nchunk - 1:
                        emit_phaseC(b)
            psB_cm.__exit__(None, None, None)

            # ---------------- Phase D: decoder ----------------------------
            psD_cm = tc.tile_pool(name="psD", bufs=1, space="PSUM")
            psD = psD_cm.__enter__()
            psT_cm = tc.tile_pool(name="psT", bufs=2, space="PSUM")
            psT = psT_cm.__enter__()
            for sw in range(NSWEEP):
                pso = [psD.tile([P, bps * P], dt.float32, tag=f"pso{m}",
                                name=f"pso{m}_{sw}")
                       for m in range(KD)]
                for c in range(nchunk):
                    if sw == 0 and c < 2:
                        wdh = wdf_pre[c]
                    else:
                        wdf = wdstage.tile([P, EK, D], dt.float32, tag="wdf",
                                           name=f"wdf{sw}_{c}")
                        nc.gpsimd.dma_start(
                            out=wdf[:, :, :],
                            in_=d_Wd[c * CHUNK:(c + 1) * CHUNK, :].rearrange(
                                "(k p) n -> p k n", p=P))
                        wdh = wdhp.tile([P, EK, D], dt.bfloat16, tag="wdh",
                                        name=f"wdh{sw}_{c}")
                        nc.vector.tensor_copy(out=wdh[:, :, :], in_=wdf[:, :, :])
                    # rebuild dense masked-e rows for this chunk + transpose
                    rhs = []
                    for es in range(EK):
                        pk = psT.tile([P, bps * P], dt.bfloat16, tag="psT",
                                      name=f"psT{sw}_{c}_{es}")
                        rhs.append((es, pk))
                    for bi in range(bps):
                        b = sw * bps + bi
                        em = emcp.tile([P, CHUNK], dt.bfloat16, tag="em")
                        nc.gpsimd.local_scatter(
                            em[:, :],
                            emcand[b][:, 16 * c:16 * c + 16],
                            lidx[b][:, 16 * c:16 * c + 16].bitcast(dt.int16),
                            channels=P, num_elems=CHUNK, num_idxs=16)
                        for (es, pk) in rhs:
                            nc.tensor.transpose(
                                pk[:, bi * P:(bi + 1) * P],
                                em[:, es * P:(es + 1) * P],
                                ident_b[:, :])
                    rr = []
                    for (es, pk) in rhs:
                        rt = rhsp.tile([P, bps * P], dt.bfloat16,
                                       tag=f"rt{es}", name=f"rt{es}_{sw}_{c}")
                        nc.scalar.copy(out=rt[:, :], in_=pk[:, :])
                        rr.append(rt)
                    for m in range(KD):
                        for es in range(EK):
                            nc.tensor.matmul(
                                pso[m][:, :],
                                wdh[:, es, m * P:(m + 1) * P],
                                rr[es][:, :],
                                start=(c == 0 and es == 0),
                                stop=(c == nchunk - 1 and es == EK - 1))
                # tail: outT slices straight from PSUM via one copy
                for m in range(KD):
                    ot = tailp.tile([P, bps * P], dt.float32, tag="ot",
                                    name=f"ot{m}_{sw}")
                    nc.scalar.copy(out=ot[:, :], in_=pso[m][:, :])
                    nc.gpsimd.dma_start(
                        out=d_outT[m * P:(m + 1) * P,
                                   sw * bps * P:(sw + 1) * bps * P],
                        in_=ot[:, :])
            psT_cm.__exit__(None, None, None)
            psD_cm.__exit__(None, None, None)

    nc.compile()
    return nc


_CACHE = {}


def _get(B_core, E):
    key = (B_core, E)
    if key not in _CACHE:
        _CACHE[key] = build(B_core, E)
    return _CACHE[key]


def kernel(x, encoder_w, encoder_b, decoder_w, k, n_cores=8):
    x = np.ascontiguousarray(np.asarray(x, dtype=np.float32))
    We = np.ascontiguousarray(np.asarray(encoder_w, dtype=np.float32))
    Wd = np.ascontiguousarray(np.asarray(decoder_w, dtype=np.float32))
    b = np.asarray(encoder_b)
    assert int(np.asarray(k)) == TOPK, f"kernel compiled for k={TOPK}"
    assert not np.any(b), "nonzero encoder_b not supported"
    B, Dd = x.shape
    E = We.shape[1]
    assert Dd == D and B % n_cores == 0
    B_core = B // n_cores

    nc = _get(B_core, E)
    in_maps = [{"x": x[i * B_core:(i + 1) * B_core], "We": We, "Wd": Wd}
               for i in range(n_cores)]
    res = bass_utils.run_bass_kernel_spmd(nc, in_maps,
                                          core_ids=list(range(n_cores)))
    return np.concatenate(
        [np.ascontiguousarray(res.results[i]["outT"].T)
         for i in range(n_cores)], axis=0)
